# revision 9
# baseline (speedup 1.0000x reference)
"""Conv2D-KAN Trainium2 kernel (8-core data-parallel SPMD), v2.

Formulation
-----------
Per 3x3 patch (N = B*30*30 patches, in_size = 288 = 9 offsets x 32 ch):
    out[n,o] = sum_{i,k} sb[n,i,k] * (spline_kernel*scale)[i,k,o]
             + silu(xf) @ scale_factor + biases
with sb an order-3 B-spline basis (8 funcs) on the uniform grid
t_r = -2.2 + 0.4 r.

Key identities:
 1. Basis values depend only on the underlying *pixel*, so features are
    computed per pixel and the op becomes a 3x3 conv with 128 filters,
    realized as shifted-view matmuls accumulating in PSUM.
 2. For a uniform grid, B_k(x) = b(v), v = (x-t_k)/h - 2, with the
    centered two-term form
        6*b(v) = relu(2-|v|)^3 - 4*relu(1-|v|)^3
    All intermediates are <= 8 (well-conditioned, bf16-safe) and the
    value is *exactly* zero outside the support — so the main matmuls
    can run fully in bf16 (1 cyc/row + fast weight load), with the /6
    folded into the weights.  Equivalent form used on-device:
        Pm = min(|v|,2)-2, Qm = min(|v|,1)-1   (in [-2,0])
        6*b = 4*Qm^3 - Pm^3
 3. The 9 per-offset silu chunks (32 rows each) are packed 4-per-matmul
    by materializing column-shifted SBUF copies of silu(x), cutting the
    matmuls per PSUM bank from 27 to 21.

Per image: 2 basis tiles [128,1024] (4 knots x 32 ch each) built with
ACT(Abs,Square,Silu) + DVE(min-sub, mul, mult-sub) + Pool(mul), then
21 bf16 matmuls per half-image PSUM bank [128 filters, 450 patches].
Each core processes 4 images; output [128, 3600] transposed on host.
"""

import sys

sys.path.insert(0, "/opt/trn_rl_repo")

import numpy as np

N_CORES = 8
B, HH, WW, C = 32, 32, 32, 32
F = 128
KH = KW = 3
HO, WO = HH - KH + 1, WW - KW + 1          # 30, 30
BPC = B // N_CORES                          # images per core = 4
PIX = HH * WW                               # 1024 pixels per image
NPC = BPC * HO * WO                         # 3600 patches per core
BANKN = 450                                 # patches per PSUM bank
HGRID = 0.4
T0 = -2.2
NCHUNK = 21                                 # matmuls per bank
# chunk k -> (kind, arg): 0..8 = B0 offsets, 9..11 = silu s0/s1/s2,
# 12..20 = B1 offsets

_cache = {}


def _build_program():
    import concourse.bacc as bacc
    import concourse.mybir as mybir
    import concourse.tile as tile

    f32 = mybir.dt.float32
    bf16 = mybir.dt.bfloat16
    AF = mybir.ActivationFunctionType
    OP = mybir.AluOpType

    nc = bacc.Bacc("TRN2", target_bir_lowering=False, debug=False)
    xt = nc.dram_tensor("xt", [C, BPC * PIX], f32, kind="ExternalInput").ap()
    wt = nc.dram_tensor("wt", [128, NCHUNK * F], bf16, kind="ExternalInput").ap()
    consts = nc.dram_tensor("consts", [128, 4], f32, kind="ExternalInput").ap()
    y = nc.dram_tensor("y", [F, NPC], f32, kind="ExternalOutput").ap()

    with tile.TileContext(nc) as tc:
        with (
            tc.tile_pool(name="wp", bufs=1) as wp,
            tc.tile_pool(name="cp", bufs=1) as cp,
            tc.tile_pool(name="xp", bufs=2) as xp,
            tc.tile_pool(name="ep", bufs=2) as ep,
            tc.tile_pool(name="bp", bufs=2) as bpool,
            tc.tile_pool(name="op", bufs=1) as op_,
            tc.tile_pool(name="pp", bufs=4, space="PSUM") as pp,
        ):
            ct = cp.tile([128, 4], f32)
            nc.scalar.dma_start(ct[:], consts[:])

            # warm the silu table set (also carries abs/square/identity)
            warm = cp.tile([1, 1], f32, tag="warm")
            nc.scalar.activation(warm[:], ct[:1, :1], AF.Silu)

            # image 0's replica DMAs first, split across two queues
            xr0 = xp.tile([128, PIX], f32, tag="xr")
            eng0 = [nc.sync, nc.scalar, nc.sync, nc.scalar]
            for rep in range(4):
                eng0[rep].dma_start(xr0[32 * rep:32 * rep + 32], xt[:, 0:PIX])

            # weights: chunks 0..11 (B0 + silu) on the scalar queue, the
            # rest on the gpsimd queue in parallel
            wA = wp.tile([128, 12 * F], bf16, tag="wA")
            nc.scalar.dma_start(wA[:], wt[:, :12 * F])
            wB = wp.tile([128, 9 * F], bf16, tag="wB")
            nc.gpsimd.dma_start(wB[:], wt[:, 12 * F:])
            wtiles = [wA[:, i * F:(i + 1) * F] for i in range(12)] + \
                     [wB[:, i * F:(i + 1) * F] for i in range(9)]

            out_t = op_.tile([F, NPC], f32)
            pending = []

            def stage1(im):
                sl = slice(im * PIX, (im + 1) * PIX)
                if im == 0:
                    xr = xr0
                else:
                    xr = xp.tile([128, PIX], f32, tag="xr", name=f"xr{im}")
                    for rep in range(4):
                        nc.sync.dma_start(
                            xr[32 * rep:32 * rep + 32], xt[:, sl])

                A0 = ep.tile([128, PIX], bf16, tag="A0", name=f"A0_{im}")
                nc.scalar.activation(A0[:], xr[:], AF.Abs,
                                     bias=ct[:, 0:1], scale=1.0 / HGRID)
                # silu(x) -> SP0[0:32] (bf16), then shifted SBUF copies
                SP0 = bpool.tile([128, PIX], bf16, tag="SP0", name=f"SP0_{im}")
                SP1 = bpool.tile([128, PIX], bf16, tag="SP1", name=f"SP1_{im}")
                nc.scalar.activation(SP0[0:32], xr[0:32], AF.Silu)
                for off in range(1, 8):
                    di, dj = divmod(off, KW)
                    s = di * WW + dj
                    dst = SP0 if off < 4 else SP1
                    g = off % 4
                    nc.sync.dma_start(
                        dst[32 * g:32 * g + 32, 0:PIX - s],
                        SP0[0:32, s:PIX])
                A1 = ep.tile([128, PIX], bf16, tag="A1", name=f"A1_{im}")
                nc.scalar.activation(A1[:], xr[:], AF.Abs,
                                     bias=ct[:, 1:2], scale=1.0 / HGRID)
                return A0, A1, SP0, SP1

            def stage2(im, st1):
                A0, A1, SP0, SP1 = st1

                def halfpipe(Ain, tag, mul_engs):
                    Pm = ep.tile([128, PIX], bf16, tag=f"P{tag}",
                                 name=f"P{tag}_{im}")
                    nc.vector.tensor_scalar(
                        Pm[:], Ain[:], 2.0, 2.0, OP.min, OP.subtract)
                    Qm = ep.tile([128, PIX], bf16, tag=f"Q{tag}",
                                 name=f"Q{tag}_{im}")
                    nc.vector.tensor_scalar(
                        Qm[:], Ain[:], 1.0, 1.0, OP.min, OP.subtract)
                    G = ep.tile([128, PIX], bf16, tag=f"G{tag}",
                                name=f"G{tag}_{im}")
                    nc.scalar.activation(G[:], Pm[:], AF.Square)
                    H = ep.tile([128, PIX], bf16, tag=f"H{tag}",
                                name=f"H{tag}_{im}")
                    nc.scalar.activation(H[:], Qm[:], AF.Square)
                    Cc = ep.tile([128, PIX], bf16, tag=f"C{tag}",
                                 name=f"C{tag}_{im}")
                    mul_engs[0].tensor_mul(Cc[:], G[:], Pm[:])
                    Dd = ep.tile([128, PIX], bf16, tag=f"D{tag}",
                                 name=f"D{tag}_{im}")
                    mul_engs[1].tensor_mul(Dd[:], H[:], Qm[:])
                    Bt = bpool.tile([128, PIX], bf16, tag=f"B{tag}",
                                    name=f"B{tag}_{im}")
                    # 6*basis = 4*Qm^3 - Pm^3  (the /6 is in the weights)
                    nc.vector.scalar_tensor_tensor(
                        Bt[:], Dd[:], 4.0, Cc[:], OP.mult, OP.subtract)
                    return Bt

                B0 = halfpipe(A0, "0", (nc.vector, nc.gpsimd))
                B1 = halfpipe(A1, "1", (nc.vector, nc.gpsimd))
                B0v = B0[:].rearrange("p (h w) -> p h w", w=WW)
                B1v = B1[:].rearrange("p (h w) -> p h w", w=WW)
                SP0v = SP0[:].rearrange("p (h w) -> p h w", w=WW)
                SP1v = SP1[:].rearrange("p (h w) -> p h w", w=WW)
                SLv = SP0[0:32].rearrange("p (h w) -> p h w", w=WW)

                # matmul order: both banks' B0 groups, then silu, then B1 —
                # late-produced tiles are consumed last.
                pss = [pp.tile([F, BANKN], f32, tag="ps", name=f"ps{im}_{h_}")
                       for h_ in range(2)]
                for half in range(2):
                    h0 = half * 15
                    ps = pss[half]
                    for k, off in enumerate(range(9)):
                        di, dj = divmod(off, KW)
                        nc.tensor.matmul(
                            ps[:], wtiles[off],
                            B0v[:, h0 + di:h0 + di + 15, dj:dj + WO],
                            start=(k == 0), stop=False)
                for half in range(2):
                    h0 = half * 15
                    ps = pss[half]
                    nc.tensor.matmul(ps[:], wtiles[9],
                                     SP0v[:, h0:h0 + 15, 0:WO],
                                     start=False, stop=False)
                    nc.tensor.matmul(ps[:], wtiles[10],
                                     SP1v[:, h0:h0 + 15, 0:WO],
                                     start=False, stop=False)
                    nc.tensor.matmul(ps[:], wtiles[11][0:32],
                                     SLv[:, h0 + 2:h0 + 17, 2:2 + WO],
                                     start=False, stop=False)
                for half in range(2):
                    h0 = half * 15
                    ps = pss[half]
                    for k, off in enumerate(range(9)):
                        di, dj = divmod(off, KW)
                        nc.tensor.matmul(
                            ps[:], wtiles[12 + off],
                            B1v[:, h0 + di:h0 + di + 15, dj:dj + WO],
                            start=False, stop=(k == 8))
                    pending.append(((im * 2 + half) * BANKN, ps))

                # drain old PSUM banks only after this image's issue
                while len(pending) > 2:
                    s, ps = pending.pop(0)
                    nc.scalar.activation(
                        out_t[:, s:s + BANKN], ps[:], AF.Identity,
                        bias=ct[:, 2:3], scale=1.0)
                    nc.sync.dma_start(y[:, s:s + BANKN], out_t[:, s:s + BANKN])

            # 2-stage pipeline: next image's input + Abs/silu issue before
            # this image's cubes/matmuls, so ACT runs ahead of the PE.
            st = stage1(0)
            for im in range(BPC):
                nxt = stage1(im + 1) if im + 1 < BPC else None
                stage2(im, st)
                st = nxt

            while pending:
                s, ps = pending.pop(0)
                nc.scalar.activation(
                    out_t[:, s:s + BANKN], ps[:], AF.Identity,
                    bias=ct[:, 2:3], scale=1.0)
                nc.sync.dma_start(y[:, s:s + BANKN], out_t[:, s:s + BANKN])

    nc.compile()
    return nc


def _prep_static(spline_kernel, scale_factor, kan_bias, conv_bias):
    import ml_dtypes

    w6 = (spline_kernel.astype(np.float64)
          * scale_factor.astype(np.float64)[:, None, :]) / 6.0
    w6r = w6.reshape(9, 32, 8, F)
    sf = scale_factor.astype(np.float64).reshape(9, 32, F)
    chunks = np.zeros((NCHUNK, 128, F), np.float64)
    for off in range(9):
        chunks[off] = w6r[off, :, 0:4].transpose(1, 0, 2).reshape(128, F)
        chunks[12 + off] = w6r[off, :, 4:8].transpose(1, 0, 2).reshape(128, F)
    for g in range(4):
        chunks[9][g * 32:(g + 1) * 32] = sf[g]
        chunks[10][g * 32:(g + 1) * 32] = sf[4 + g]
    chunks[11][0:32] = sf[8]
    wtc = np.ascontiguousarray(
        chunks.transpose(1, 0, 2).reshape(128, NCHUNK * F))
    wt = wtc.astype(ml_dtypes.bfloat16)

    consts = np.zeros((128, 4), np.float32)
    kl = np.arange(128) // 32
    # v = x/h - k + 3.5  (k = knot index); tile0 k = kl, tile1 k = 4+kl
    consts[:, 0] = 3.5 - kl
    consts[:, 1] = 3.5 - (4 + kl)
    consts[:, 2] = (kan_bias.astype(np.float64)
                    + conv_bias.astype(np.float64)).astype(np.float32)
    return wt, consts


def kernel(x, spline_kernel, scale_factor, kan_bias, conv_bias):
    from concourse import bass_utils

    x = np.asarray(x, np.float32)
    spline_kernel = np.asarray(spline_kernel, np.float32)
    scale_factor = np.asarray(scale_factor, np.float32)
    kan_bias = np.asarray(kan_bias, np.float32)
    conv_bias = np.asarray(conv_bias, np.float32)

    if "nc" not in _cache:
        _cache["nc"] = _build_program()
    nc = _cache["nc"]

    wt, consts = _prep_static(spline_kernel, scale_factor,
                              kan_bias, conv_bias)

    in_maps = []
    for c in range(N_CORES):
        xc = x[c * BPC:(c + 1) * BPC]                      # (4,32,32,32)
        xtc = np.ascontiguousarray(
            xc.transpose(3, 0, 1, 2).reshape(C, BPC * PIX), np.float32
        )
        in_maps.append({"xt": xtc, "wt": wt, "consts": consts})

    res = bass_utils.run_bass_kernel_spmd(
        nc, in_maps, core_ids=list(range(N_CORES)),
        **_cache.get("run_kwargs", {})
    )
    _cache["last_result"] = res

    out = np.empty((B, HO, WO, F), np.float32)
    for c in range(N_CORES):
        yc = res.results[c]["y"]                           # (128, 3600)
        out[c * BPC:(c + 1) * BPC] = (
            yc.reshape(F, BPC, HO, WO).transpose(1, 2, 3, 0)
        )
    return out


# revision 10
# speedup vs baseline: 1.0689x; 1.0689x over previous
"""Conv2D-KAN Trainium2 kernel (8-core data-parallel SPMD), v2.

Formulation
-----------
Per 3x3 patch (N = B*30*30 patches, in_size = 288 = 9 offsets x 32 ch):
    out[n,o] = sum_{i,k} sb[n,i,k] * (spline_kernel*scale)[i,k,o]
             + silu(xf) @ scale_factor + biases
with sb an order-3 B-spline basis (8 funcs) on the uniform grid
t_r = -2.2 + 0.4 r.

Key identities:
 1. Basis values depend only on the underlying *pixel*, so features are
    computed per pixel and the op becomes a 3x3 conv with 128 filters,
    realized as shifted-view matmuls accumulating in PSUM.
 2. For a uniform grid, B_k(x) = b(v), v = (x-t_k)/h - 2, with the
    centered two-term form
        6*b(v) = relu(2-|v|)^3 - 4*relu(1-|v|)^3
    All intermediates are <= 8 (well-conditioned, bf16-safe) and the
    value is *exactly* zero outside the support — so the main matmuls
    can run fully in bf16 (1 cyc/row + fast weight load), with the /6
    folded into the weights.  Equivalent form used on-device:
        Pm = min(|v|,2)-2, Qm = min(|v|,1)-1   (in [-2,0])
        6*b = 4*Qm^3 - Pm^3
 3. The 9 per-offset silu chunks (32 rows each) are packed 4-per-matmul
    by materializing column-shifted SBUF copies of silu(x), cutting the
    matmuls per PSUM bank from 27 to 21.

Per image: 2 basis tiles [128,1024] (4 knots x 32 ch each) built with
ACT(Abs,Square,Silu) + DVE(min-sub, mul, mult-sub) + Pool(mul), then
21 bf16 matmuls per half-image PSUM bank [128 filters, 450 patches].
Each core processes 4 images; output [128, 3600] transposed on host.
"""

import sys

sys.path.insert(0, "/opt/trn_rl_repo")

import numpy as np

N_CORES = 8
B, HH, WW, C = 32, 32, 32, 32
F = 128
KH = KW = 3
HO, WO = HH - KH + 1, WW - KW + 1          # 30, 30
BPC = B // N_CORES                          # images per core = 4
PIX = HH * WW                               # 1024 pixels per image
NPC = BPC * HO * WO                         # 3600 patches per core
BANKN = 450                                 # patches per PSUM bank
HGRID = 0.4
T0 = -2.2
NCHUNK = 21                                 # matmuls per bank
# chunk k -> (kind, arg): 0..8 = B0 offsets, 9..11 = silu s0/s1/s2,
# 12..20 = B1 offsets

_cache = {}


def _build_program():
    import concourse.bacc as bacc
    import concourse.mybir as mybir
    import concourse.tile as tile

    f32 = mybir.dt.float32
    bf16 = mybir.dt.bfloat16
    AF = mybir.ActivationFunctionType
    OP = mybir.AluOpType

    nc = bacc.Bacc("TRN2", target_bir_lowering=False, debug=False)
    xt = nc.dram_tensor("xt", [C, BPC * PIX], f32, kind="ExternalInput").ap()
    wt = nc.dram_tensor("wt", [128, NCHUNK * F], bf16, kind="ExternalInput").ap()
    consts = nc.dram_tensor("consts", [128, 4], f32, kind="ExternalInput").ap()
    y = nc.dram_tensor("y", [F, NPC], f32, kind="ExternalOutput").ap()

    with tile.TileContext(nc) as tc:
        with (
            tc.tile_pool(name="wp", bufs=1) as wp,
            tc.tile_pool(name="cp", bufs=1) as cp,
            tc.tile_pool(name="xp", bufs=2) as xp,
            tc.tile_pool(name="ep", bufs=2) as ep,
            tc.tile_pool(name="bp", bufs=2) as bpool,
            tc.tile_pool(name="op", bufs=1) as op_,
            tc.tile_pool(name="pp", bufs=4, space="PSUM") as pp,
        ):
            ct = cp.tile([128, 4], f32)
            nc.scalar.dma_start(ct[:], consts[:])

            # warm the silu table set (also carries abs/square/identity)
            warm = cp.tile([1, 1], f32, tag="warm")
            nc.scalar.activation(warm[:], ct[:1, :1], AF.Silu)

            # image 0's replica DMAs first, split across two queues
            xr0 = xp.tile([128, PIX], f32, tag="xr")
            eng0 = [nc.sync, nc.scalar, nc.sync, nc.scalar]
            for rep in range(4):
                eng0[rep].dma_start(xr0[32 * rep:32 * rep + 32], xt[:, 0:PIX])

            # weights: chunks 0..11 (B0 + silu) on the scalar queue, the
            # rest on the gpsimd queue in parallel
            wA = wp.tile([128, 12 * F], bf16, tag="wA")
            nc.scalar.dma_start(wA[:], wt[:, :12 * F])
            wB = wp.tile([128, 9 * F], bf16, tag="wB")
            nc.gpsimd.dma_start(wB[:], wt[:, 12 * F:])
            wtiles = [wA[:, i * F:(i + 1) * F] for i in range(12)] + \
                     [wB[:, i * F:(i + 1) * F] for i in range(9)]

            out_t = op_.tile([F, NPC], f32)
            pending = []

            def stage1(im):
                sl = slice(im * PIX, (im + 1) * PIX)
                if im == 0:
                    xr = xr0
                else:
                    xr = xp.tile([128, PIX], f32, tag="xr", name=f"xr{im}")
                    for rep in range(4):
                        nc.sync.dma_start(
                            xr[32 * rep:32 * rep + 32], xt[:, sl])

                A0 = ep.tile([128, PIX], bf16, tag="A0", name=f"A0_{im}")
                nc.scalar.activation(A0[:], xr[:], AF.Abs,
                                     bias=ct[:, 0:1], scale=1.0 / HGRID)
                # silu(x) -> SP0[0:32] (bf16), then shifted SBUF copies
                SP0 = bpool.tile([128, PIX], bf16, tag="SP0", name=f"SP0_{im}")
                SP1 = bpool.tile([128, PIX], bf16, tag="SP1", name=f"SP1_{im}")
                nc.scalar.activation(SP0[0:32], xr[0:32], AF.Silu)
                for off in range(1, 8):
                    di, dj = divmod(off, KW)
                    s = di * WW + dj
                    dst = SP0 if off < 4 else SP1
                    g = off % 4
                    nc.sync.dma_start(
                        dst[32 * g:32 * g + 32, 0:PIX - s],
                        SP0[0:32, s:PIX])
                A1 = ep.tile([128, PIX], bf16, tag="A1", name=f"A1_{im}")
                nc.scalar.activation(A1[:], xr[:], AF.Abs,
                                     bias=ct[:, 1:2], scale=1.0 / HGRID)
                return A0, A1, SP0, SP1

            def stage2(im, st1):
                A0, A1, SP0, SP1 = st1

                def halfpipe(Ain, tag, mul_engs):
                    Pm = ep.tile([128, PIX], bf16, tag=f"P{tag}",
                                 name=f"P{tag}_{im}")
                    nc.vector.tensor_scalar(
                        Pm[:], Ain[:], 2.0, 2.0, OP.min, OP.subtract)
                    Qm = ep.tile([128, PIX], bf16, tag=f"Q{tag}",
                                 name=f"Q{tag}_{im}")
                    nc.vector.tensor_scalar(
                        Qm[:], Ain[:], 1.0, 1.0, OP.min, OP.subtract)
                    G = ep.tile([128, PIX], bf16, tag=f"G{tag}",
                                name=f"G{tag}_{im}")
                    nc.scalar.activation(G[:], Pm[:], AF.Square)
                    H = ep.tile([128, PIX], bf16, tag=f"H{tag}",
                                name=f"H{tag}_{im}")
                    nc.scalar.activation(H[:], Qm[:], AF.Square)
                    Cc = ep.tile([128, PIX], bf16, tag=f"C{tag}",
                                 name=f"C{tag}_{im}")
                    mul_engs[0].tensor_mul(Cc[:], G[:], Pm[:])
                    Dd = ep.tile([128, PIX], bf16, tag=f"D{tag}",
                                 name=f"D{tag}_{im}")
                    mul_engs[1].tensor_mul(Dd[:], H[:], Qm[:])
                    Bt = bpool.tile([128, PIX], bf16, tag=f"B{tag}",
                                    name=f"B{tag}_{im}")
                    # 6*basis = 4*Qm^3 - Pm^3  (the /6 is in the weights)
                    nc.vector.scalar_tensor_tensor(
                        Bt[:], Dd[:], 4.0, Cc[:], OP.mult, OP.subtract)
                    return Bt

                B0 = halfpipe(A0, "0", (nc.vector, nc.gpsimd))
                B1 = halfpipe(A1, "1", (nc.vector, nc.gpsimd))
                B0v = B0[:].rearrange("p (h w) -> p h w", w=WW)
                B1v = B1[:].rearrange("p (h w) -> p h w", w=WW)
                SP0v = SP0[:].rearrange("p (h w) -> p h w", w=WW)
                SP1v = SP1[:].rearrange("p (h w) -> p h w", w=WW)
                SLv = SP0[0:32].rearrange("p (h w) -> p h w", w=WW)

                # matmul order: both banks' B0 groups, then silu, then B1 —
                # late-produced tiles are consumed last.
                pss = [pp.tile([F, BANKN], f32, tag="ps", name=f"ps{im}_{h_}")
                       for h_ in range(2)]
                for half in range(2):
                    h0 = half * 15
                    ps = pss[half]
                    for k, off in enumerate(range(9)):
                        di, dj = divmod(off, KW)
                        nc.tensor.matmul(
                            ps[:], wtiles[off],
                            B0v[:, h0 + di:h0 + di + 15, dj:dj + WO],
                            start=(k == 0), stop=False)
                for half in range(2):
                    h0 = half * 15
                    ps = pss[half]
                    nc.tensor.matmul(ps[:], wtiles[9],
                                     SP0v[:, h0:h0 + 15, 0:WO],
                                     start=False, stop=False)
                    nc.tensor.matmul(ps[:], wtiles[10],
                                     SP1v[:, h0:h0 + 15, 0:WO],
                                     start=False, stop=False)
                    nc.tensor.matmul(ps[:], wtiles[11][0:32],
                                     SLv[:, h0 + 2:h0 + 17, 2:2 + WO],
                                     start=False, stop=False)
                for half in range(2):
                    h0 = half * 15
                    ps = pss[half]
                    for k, off in enumerate(range(9)):
                        di, dj = divmod(off, KW)
                        nc.tensor.matmul(
                            ps[:], wtiles[12 + off],
                            B1v[:, h0 + di:h0 + di + 15, dj:dj + WO],
                            start=False, stop=(k == 8))
                    pending.append(((im * 2 + half) * BANKN, ps))

                # drain old PSUM banks only after this image's issue
                while len(pending) > 2:
                    s, ps = pending.pop(0)
                    nc.scalar.activation(
                        out_t[:, s:s + BANKN], ps[:], AF.Identity,
                        bias=ct[:, 2:3], scale=1.0)
                    nc.sync.dma_start(y[:, s:s + BANKN], out_t[:, s:s + BANKN])

            # sequential per-image issue; ACT runs ahead via deferred outs
            for im in range(BPC):
                st = stage1(im)
                stage2(im, st)

            while pending:
                s, ps = pending.pop(0)
                nc.scalar.activation(
                    out_t[:, s:s + BANKN], ps[:], AF.Identity,
                    bias=ct[:, 2:3], scale=1.0)
                nc.sync.dma_start(y[:, s:s + BANKN], out_t[:, s:s + BANKN])

    nc.compile()
    return nc


def _prep_static(spline_kernel, scale_factor, kan_bias, conv_bias):
    import ml_dtypes

    w6 = (spline_kernel.astype(np.float64)
          * scale_factor.astype(np.float64)[:, None, :]) / 6.0
    w6r = w6.reshape(9, 32, 8, F)
    sf = scale_factor.astype(np.float64).reshape(9, 32, F)
    chunks = np.zeros((NCHUNK, 128, F), np.float64)
    for off in range(9):
        chunks[off] = w6r[off, :, 0:4].transpose(1, 0, 2).reshape(128, F)
        chunks[12 + off] = w6r[off, :, 4:8].transpose(1, 0, 2).reshape(128, F)
    for g in range(4):
        chunks[9][g * 32:(g + 1) * 32] = sf[g]
        chunks[10][g * 32:(g + 1) * 32] = sf[4 + g]
    chunks[11][0:32] = sf[8]
    wtc = np.ascontiguousarray(
        chunks.transpose(1, 0, 2).reshape(128, NCHUNK * F))
    wt = wtc.astype(ml_dtypes.bfloat16)

    consts = np.zeros((128, 4), np.float32)
    kl = np.arange(128) // 32
    # v = x/h - k + 3.5  (k = knot index); tile0 k = kl, tile1 k = 4+kl
    consts[:, 0] = 3.5 - kl
    consts[:, 1] = 3.5 - (4 + kl)
    consts[:, 2] = (kan_bias.astype(np.float64)
                    + conv_bias.astype(np.float64)).astype(np.float32)
    return wt, consts


def kernel(x, spline_kernel, scale_factor, kan_bias, conv_bias):
    from concourse import bass_utils

    x = np.asarray(x, np.float32)
    spline_kernel = np.asarray(spline_kernel, np.float32)
    scale_factor = np.asarray(scale_factor, np.float32)
    kan_bias = np.asarray(kan_bias, np.float32)
    conv_bias = np.asarray(conv_bias, np.float32)

    if "nc" not in _cache:
        _cache["nc"] = _build_program()
    nc = _cache["nc"]

    wt, consts = _prep_static(spline_kernel, scale_factor,
                              kan_bias, conv_bias)

    in_maps = []
    for c in range(N_CORES):
        xc = x[c * BPC:(c + 1) * BPC]                      # (4,32,32,32)
        xtc = np.ascontiguousarray(
            xc.transpose(3, 0, 1, 2).reshape(C, BPC * PIX), np.float32
        )
        in_maps.append({"xt": xtc, "wt": wt, "consts": consts})

    res = bass_utils.run_bass_kernel_spmd(
        nc, in_maps, core_ids=list(range(N_CORES)),
        **_cache.get("run_kwargs", {})
    )
    _cache["last_result"] = res

    out = np.empty((B, HO, WO, F), np.float32)
    for c in range(N_CORES):
        yc = res.results[c]["y"]                           # (128, 3600)
        out[c * BPC:(c + 1) * BPC] = (
            yc.reshape(F, BPC, HO, WO).transpose(1, 2, 3, 0)
        )
    return out
